# revision 24
# baseline (speedup 1.0000x reference)
"""MemSAC loss (retrieval kNN + masked log-softmax contrastive loss) on 8 Trainium2 cores.

Sharding: the 48000-slot memory queue is split 6000 rows/core (padded to
6016 = 47*128, pad rows zero / label 126). Queue rows are pre-sorted by
label per shard (host-side layout permutation) so a row's label is a step
function of its index, recoverable with compare-accumulate against a
126-entry boundary table.

Per core:
  - normalize targets (f32) -> bf16, PE-transpose to [D, T]
  - queue norms via fused multiply+accumulate; inverse norms fold into
    (a) diagonal-scaled PE transposes that produce the normalized
        transposed queue [D, Q] directly from the raw bf16 shard, and
    (b) the one-hot matrix (onehotn = onehot * invnorm) used for class sums
  - sim matmuls (bf16) into [128-target, 1024-col] PSUM superblocks; one
    ACT pass per superblock computes exp(sim/tau) (bf16 out) + row-sum
  - gpsimd packs (bf16-bits << 13 | column-index) into an int32 array whose
    f32 bit pattern is order-isomorphic to sim; one DVE max8 per target
    chunk yields the top-8 values AND indices (no max_index pass)
  - labels decoded from indices, candidates packed as
    (bf16bits << 7 | 127-label) -- exact in fp32
  - class sums W = qraw^T @ onehotn and counts via PE accumulation groups;
    S = tgt_n @ W^T
  - one AllReduce combines candidates (per-core slices of a zeroed
    buffer), S, sum-exp, counts
  - every core redundantly merges 64 candidates -> global top-5 ->
    majority vote (pairwise-equality count, score = cnt*1024+1023-label)
    -> pseudo label -> S[t,pseudo]/cnt[pseudo] -> loss.

PSUM accumulation groups are kept strictly contiguous on the PE via an
explicit instruction chain: the PE crashes (NRT_EXEC_UNIT_UNRECOVERABLE)
if two accumulation groups interleave.

kernel() takes FULL unsharded inputs and returns the FULL scalar output.
"""

import os
import sys

sys.path.insert(0, "/opt/trn_rl_repo")
os.environ.setdefault("MYCRO_LOCAL_CACHE", "1")

import numpy as np
from contextlib import ExitStack

import concourse.bass as bass
import concourse.bacc as bacc
import concourse.tile as tile
from concourse.tile import add_dep_helper
from concourse import mybir
from concourse.bass_utils import run_bass_kernel_spmd

AF = mybir.ActivationFunctionType
AL = mybir.AluOpType
AX = mybir.AxisListType
F32 = mybir.dt.float32
BF16 = mybir.dt.bfloat16
I32 = mybir.dt.int32
U16 = mybir.dt.uint16
NP_BF16 = mybir.dt.np(mybir.dt.bfloat16)

SKIP_GC = False
# ---- problem constants ----
D = 256
Q = 48000
C = 126          # n classes
BS = 512         # source batch
BT = 512         # target batch
TAU = 0.07
COEFF = 0.1
WARM_UP = 4000
NCORES = 8
QS = Q // NCORES            # 6000 real rows per core
NT = (QS + 127) // 128      # 47 tiles
QSP = NT * 128              # 6016 padded rows per core
NPAD = QSP - QS             # 16 pad rows per core
CAND_N = 8                  # candidates kept per core per row
SB = 1024                   # superblock width (2 PSUM banks)


def build_program(n_cores, qsp, bt, n_pad_per_core=None, stage=8,
                  mock_cc=False, n_reps=1):
    """Build the SPMD Bass program. Identical NEFF runs on all cores."""
    if n_pad_per_core is None:
        n_pad_per_core = NPAD
    nt = qsp // 128
    tcn = bt // 128
    nsb = (qsp + SB - 1) // SB
    sz_cand = n_cores * bt * CAND_N
    sz_s = bt * C
    off_s = sz_cand
    off_se = off_s + sz_s
    off_cnt = off_se + bt
    rb = off_cnt + C
    rbp = ((rb + 255) // 256) * 256
    inv_tau = 1.0 / TAU

    nc = bacc.Bacc("TRN2", target_bir_lowering=False, debug=False,
                   num_devices=n_cores)

    tgt_d = nc.dram_tensor("tgt", [bt, D], F32, kind="ExternalInput")
    qsh_d = nc.dram_tensor("qshard", [qsp, D], BF16, kind="ExternalInput")
    labT_d = nc.dram_tensor("labT", [128, nt], F32, kind="ExternalInput")
    bnd_d = nc.dram_tensor("bounds", [C], F32, kind="ExternalInput")
    it_d = nc.dram_tensor("itv", [1, 1], F32, kind="ExternalInput")
    out_d = nc.dram_tensor("outv", [1, 1], F32, kind="ExternalOutput")
    red_in = nc.dram_tensor("red_in", [rbp], F32)
    red_out = nc.dram_tensor("red_out", [rbp], F32,
                             addr_space="Shared" if n_cores > 4 else "Local")

    with tile.TileContext(nc) as tc:
        def _emit(ctx):
            sb = ctx.enter_context(tc.tile_pool(name="sb", bufs=1))
            sb2 = ctx.enter_context(tc.tile_pool(name="sb2", bufs=2))
            sb3 = ctx.enter_context(tc.tile_pool(name="sb3", bufs=3))
            psA = ctx.enter_context(tc.tile_pool(name="psA", bufs=2,
                                                 space="PSUM"))
            psW = ctx.enter_context(tc.tile_pool(name="psW", bufs=1,
                                                 space="PSUM"))
            psS = ctx.enter_context(tc.tile_pool(name="psS", bufs=1,
                                                 space="PSUM"))

            # PE group-contiguity chain (see module docstring)
            _pe_prev = [None]

            def pe(bi):
                if _pe_prev[0] is not None:
                    add_dep_helper(bi.ins, _pe_prev[0].ins, sync=False,
                                   reason="PE group contiguity")
                _pe_prev[0] = bi
                return bi

            def rsqrt_dve(dst, x, pool, w):
                # y = 1/sqrt(x) via 0x5f3759df magic + 2 Newton steps.
                # Tiny [128, w] tiles; keeps Sqrt out of the ACT table so
                # the exp set stays resident for the whole sim phase.
                xc = pool.tile([128, w], F32, tag="rsx", name="rsx",
                               bufs=4)
                nc.vector.tensor_scalar(xc[:], x, 1e-12, None, AL.max)
                x = xc[:]
                t0 = pool.tile([128, w], I32, tag="rs0", name="rs0", bufs=4)
                nc.vector.tensor_scalar(t0[:], x.bitcast(I32), 1, None,
                                        AL.logical_shift_right)
                y0 = pool.tile([128, w], I32, tag="rs1", name="rs1", bufs=4)
                nc.vector.tensor_scalar(y0[:], t0[:], -1, 0x5f3759df,
                                        AL.mult, AL.add)
                y = y0[:].bitcast(F32)
                for _ in range(2):
                    t1 = pool.tile([128, w], F32, tag="rs2", name="rs2",
                                   bufs=4)
                    nc.vector.tensor_mul(t1[:], y, y)
                    nc.vector.tensor_mul(t1[:], t1[:], x)
                    nc.vector.tensor_scalar(t1[:], t1[:], -0.5, 1.5,
                                            AL.mult, AL.add)
                    yn = pool.tile([128, w], F32, tag="rs3", name="rs3",
                                   bufs=4)
                    nc.vector.tensor_mul(yn[:], y, t1[:])
                    y = yn[:]
                nc.vector.tensor_copy(dst, y)

            # ---------- constants ----------
            identi = sb.tile([128, 128], I32, tag="identi")
            nc.gpsimd.iota(identi[:], pattern=[[-1, 128]], base=0,
                           channel_multiplier=1)
            ident = sb.tile([128, 128], BF16, tag="ident")
            nc.vector.tensor_scalar(ident[:], identi[:], 0, None, AL.is_equal)

            ciota_i = sb.tile([128, C], I32, tag="ciota_i")
            nc.gpsimd.iota(ciota_i[:], pattern=[[1, C]], base=0,
                           channel_multiplier=0)
            ciota = sb.tile([128, C], F32, tag="ciota")
            nc.vector.tensor_copy(ciota[:], ciota_i[:])

            ones_bf = sb.tile([128, 1], BF16, tag="ones_bf")
            nc.vector.memset(ones_bf[:], 1.0)
            ones_f = sb.tile([128, 1], F32, tag="ones_f")
            nc.vector.memset(ones_f[:], 1.0)
            eps = sb.tile([128, 1], F32, tag="eps")
            nc.vector.memset(eps[:], 1e-24)
            zeros_big = sb.tile([128, max(sz_cand // 128, 256)], F32,
                                tag="zeros")
            nc.vector.memset(zeros_big[:], 0.0)

            e_bc = sb.tile([128, C], F32, tag="e_bc")
            nc.sync.dma_start(
                out=e_bc[:],
                in_=bnd_d.ap().unsqueeze(0).partition_broadcast(128))

            # ---------- target prep ----------
            tgtf = sb.tile([128, tcn, D], F32, tag="tgtf")
            nc.sync.dma_start(
                out=tgtf[:],
                in_=tgt_d.ap().rearrange("(c p) d -> p c d", p=128))
            ssq_t = sb.tile([128, tcn], F32, tag="ssq_t")
            sqscr = sb.tile([128, D], F32, tag="sqscr")
            for c_ in range(tcn):
                nc.vector.scalar_tensor_tensor(
                    out=sqscr[:], in0=tgtf[:, c_], scalar=1.0,
                    in1=tgtf[:, c_], op0=AL.bypass, op1=AL.mult,
                    accum_out=ssq_t[:, c_:c_ + 1])
            inv_t = sb.tile([128, tcn], F32, tag="inv_t")
            rsqrt_dve(inv_t[:], ssq_t[:], sb3, tcn)
            tgtn = sb.tile([128, tcn, D], BF16, tag="tgtn")
            for c_ in range(tcn):
                nc.scalar.activation(tgtn[:, c_], tgtf[:, c_], AF.Copy,
                                     bias=0.0, scale=inv_t[:, c_:c_ + 1])

            tgtT = [sb.tile([128, bt], BF16, tag=f"tgtT{kh}",
                            name=f"tgtT{kh}") for kh in range(2)]
            for c_ in range(tcn):
                for kh in range(2):
                    pt = psA.tile([128, 128], BF16, tag="mm", name="pt")
                    pe(nc.tensor.transpose(
                        pt[:, 0:128], tgtn[:, c_, kh * 128:(kh + 1) * 128],
                        ident[:]))
                    nc.scalar.activation(
                        tgtT[kh][:, c_ * 128:(c_ + 1) * 128], pt[:, 0:128],
                        AF.Copy)

            # ---------- queue shard load + norms ----------
            labT = sb.tile([128, nt], F32, tag="labT")
            nc.sync.dma_start(out=labT[:], in_=labT_d.ap())
            qsb = sb.tile([128, nt, D], BF16, tag="qsb")
            qview = qsh_d.ap().rearrange("(t p) d -> p t d", p=128)
            ssq_q = sb.tile([128, nt], F32, tag="ssq_q")
            sqscr2 = sb.tile([128, D], F32, tag="sqscr2")
            sqscr3 = sb.tile([128, D], BF16, tag="sqscr3")
            inv_q = sb.tile([128, nt], F32, tag="inv_q")
            oh_all = sb.tile([128, nt, C], BF16, tag="oh_all")
            qT = [sb.tile([128, qsp], BF16, tag=f"qT{kh}", name=f"qT{kh}")
                  for kh in range(2)]
            ngr = (nt + 3) // 4

            def prep_group(g):
                # load 4 tiles, norms for them, one-hot, scaled transposes
                ts0 = g * 4
                tn = min(4, nt - ts0)
                nc.sync.dma_start(out=qsb[:, ts0:ts0 + tn],
                                  in_=qview[:, ts0:ts0 + tn])
                for j in range(tn):
                    t = ts0 + j
                    if t % 2 == 0:
                        nc.vector.scalar_tensor_tensor(
                            out=sqscr2[:], in0=qsb[:, t], scalar=1.0,
                            in1=qsb[:, t], op0=AL.bypass, op1=AL.mult,
                            accum_out=ssq_q[:, t:t + 1])
                    else:
                        nc.scalar.activation(sqscr3[:], qsb[:, t],
                                             AF.Square,
                                             accum_out=ssq_q[:, t:t + 1])
                rsqrt_dve(inv_q[:, ts0:ts0 + tn],
                          ssq_q[:, ts0:ts0 + tn], sb3, tn)
                dgs = []
                for j in range(tn):
                    t = ts0 + j
                    nc.vector.tensor_scalar(oh_all[:, t], ciota[:],
                                            labT[:, t:t + 1],
                                            inv_q[:, t:t + 1],
                                            AL.is_equal, AL.mult)
                    dg = sb3.tile([128, 128], BF16, tag="diag",
                                  name="dg", bufs=8)
                    nc.vector.tensor_scalar(
                        dg[:], identi[:], 0, inv_q[:, t:t + 1],
                        AL.is_equal, AL.mult)
                    dgs.append(dg)
                for kh in range(2):
                    pq = psA.tile([128, 512], F32, tag="mm", name="pq")
                    for j in range(tn):
                        t = ts0 + j
                        pe(nc.tensor.matmul(
                            pq[:, j * 128:(j + 1) * 128],
                            lhsT=qsb[:, t, kh * 128:(kh + 1) * 128],
                            rhs=dgs[j][:], start=(j == 0),
                            stop=(j == tn - 1),
                            skip_group_check=SKIP_GC))
                    if kh == 0:
                        nc.scalar.activation(
                            qT[kh][:, ts0 * 128:(ts0 + tn) * 128],
                            pq[:, 0:tn * 128], AF.Copy)
                    else:
                        nc.vector.tensor_copy(
                            qT[kh][:, ts0 * 128:(ts0 + tn) * 128],
                            pq[:, 0:tn * 128])

            if stage >= 3:
                se_all = sb.tile([128, tcn], F32, tag="se_all")
                packs = sb.tile([128, tcn, CAND_N], F32, tag="packs")

                def emit_tc(tci, interleave_prep):
                    # vp[p, q] int32 = (bf16bits(exp(sim/tau)) << 16) | q :
                    # iota fills the low u16 halves (gpsimd), the ACT exp
                    # writes its bf16 output directly into the high halves.
                    # As f32 bit patterns these are positive, NaN-free, and
                    # ordered exactly like sim -- one max8 gives the top-8
                    # values AND their column indices.
                    vp = sb2.tile([128, qsp], I32, tag="vp", name="vp")
                    vp_u16 = vp[:].bitcast(U16).rearrange(
                        "p (q two) -> p q two", two=2)
                    nc.gpsimd.iota(vp_u16[:, :, 0], pattern=[[1, qsp]],
                                   base=0, channel_multiplier=0)
                    vp_hi = vp[:].bitcast(BF16).rearrange(
                        "p (q two) -> p q two", two=2)[:, :, 1]
                    parts = sb2.tile([128, nsb], F32, tag="parts",
                                     name="parts")
                    for sbi in range(nsb):
                        if interleave_prep:
                            while _prep_done[0] * 128 * 4 < min(
                                    (sbi + 1) * SB, nt * 128) or                                     (sbi == nsb - 1 and
                                     _prep_done[0] < ngr):
                                prep_group(_prep_done[0])
                                _prep_done[0] += 1
                        w = min(SB, qsp - sbi * SB)
                        ps = psA.tile([128, SB], F32, tag="mm", name="ps")
                        for half in range(0, w, 512):
                            n = min(512, w - half)
                            for kh in range(2):
                                col = sbi * SB + half
                                pe(nc.tensor.matmul(
                                    ps[:, half:half + n],
                                    lhsT=tgtT[kh][:, tci * 128:
                                                  (tci + 1) * 128],
                                    rhs=qT[kh][:, col:col + n],
                                    start=(kh == 0), stop=(kh == 1),
                                    skip_group_check=SKIP_GC))
                        nc.scalar.activation(
                            vp_hi[:, sbi * SB:sbi * SB + w], ps[:, 0:w],
                            AF.Exp, scale=inv_tau,
                            accum_out=parts[:, sbi:sbi + 1])
                    nc.vector.reduce_sum(se_all[:, tci:tci + 1], parts[:],
                                         axis=AX.X)
                    vp8 = sb2.tile([128, 8], F32, tag="vp8", name="vp8")
                    nc.vector.max(vp8[:], vp[:].bitcast(F32))
                    vp8i = vp8[:].bitcast(I32)
                    gix = sb2.tile([128, 8], I32, tag="gix", name="gix")
                    nc.vector.tensor_scalar(gix[:], vp8i, 65535, None,
                                            AL.bitwise_and)
                    gixf = sb2.tile([128, 8], F32, tag="gixf", name="gixf")
                    nc.vector.tensor_copy(gixf[:], gix[:])
                    lab8 = sb2.tile([128, 8], F32, tag="lab8", name="lab8")
                    scr126 = sb2.tile([128, C], F32, tag="scr126",
                                      name="scr126")
                    for k in range(8):
                        nc.vector.scalar_tensor_tensor(
                            out=scr126[:], in0=e_bc[:],
                            scalar=gixf[:, k:k + 1], in1=e_bc[:],
                            op0=AL.is_le, op1=AL.bypass,
                            accum_out=lab8[:, k:k + 1])
                    sh2 = sb2.tile([128, 8], I32, tag="sh2", name="sh2")
                    nc.vector.tensor_scalar(sh2[:], vp8i, 16, 7,
                                            AL.logical_shift_right,
                                            AL.logical_shift_left)
                    lab8i = sb2.tile([128, 8], I32, tag="lab8i",
                                     name="lab8i")
                    nc.vector.tensor_copy(lab8i[:], lab8[:])
                    packi = sb2.tile([128, 8], I32, tag="packi",
                                     name="packi")
                    nc.vector.scalar_tensor_tensor(
                        out=packi[:], in0=sh2[:], scalar=127, in1=lab8i[:],
                        op0=AL.add, op1=AL.subtract)
                    nc.vector.tensor_copy(packs[:, tci], packi[:])

                _prep_done = [0]
                for tci in range(tcn):
                    emit_tc(tci, interleave_prep=(tci == 0))

            if stage >= 4:
                # ------- class sums W / counts (contiguous groups) --------
                nrm_bf = sb.tile([128, nt], BF16, tag="nrm_bf")
                nc.vector.tensor_mul(nrm_bf[:], ssq_q[:], inv_q[:])
                wt_ps = [psW.tile([128, C], F32, tag=f"wt{kh}",
                                  name=f"wtps{kh}") for kh in range(2)]
                cnt_ps = psW.tile([1, C], F32, tag="cnt", name="cntps")
                for kh in range(2):
                    for t in range(nt):
                        pe(nc.tensor.matmul(
                            wt_ps[kh][:],
                            lhsT=qsb[:, t, kh * 128:(kh + 1) * 128],
                            rhs=oh_all[:, t], start=(t == 0),
                            stop=(t == nt - 1), skip_group_check=SKIP_GC))
                for t in range(nt):
                    pe(nc.tensor.matmul(
                        cnt_ps[:], lhsT=nrm_bf[:, t:t + 1],
                        rhs=oh_all[:, t], start=(t == 0),
                        stop=(t == nt - 1), skip_group_check=SKIP_GC))

                # ---------- per-class sums S = tgt_n @ W.T ----------
                wt_sb = [sb.tile([128, C], BF16, tag=f"wtsb{kh}",
                                 name=f"wtsb{kh}") for kh in range(2)]
                for kh in range(2):
                    nc.scalar.activation(wt_sb[kh][:], wt_ps[kh][:], AF.Copy)
                for tci in range(tcn):
                    pS = psS.tile([128, C], F32, tag="s", name="pS")
                    for kh in range(2):
                        pe(nc.tensor.matmul(
                            pS[:],
                            lhsT=tgtT[kh][:, tci * 128:(tci + 1) * 128],
                            rhs=wt_sb[kh][:], start=(kh == 0),
                            stop=(kh == 1), skip_group_check=SKIP_GC))
                    sout = sb3.tile([128, C], F32, tag="sout", name="sout")
                    nc.scalar.activation(sout[:], pS[:], AF.Copy)
                    nc.sync.dma_start(
                        out=red_in.ap()[off_s + tci * 128 * C:
                                        off_s + (tci + 1) * 128 * C]
                        .rearrange("(p c) -> p c", p=128),
                        in_=sout[:])

            if stage >= 5:
                # ---------- assemble reduce buffer ----------
                cnt_sb0 = sb.tile([1, C], F32, tag="cnt_sb0")
                nc.scalar.activation(cnt_sb0[:], cnt_ps[:], AF.Copy)
                nc.sync.dma_start(
                    out=red_in.ap()[off_cnt:off_cnt + C].unsqueeze(0),
                    in_=cnt_sb0[:])
                nc.sync.dma_start(
                    out=red_in.ap()[off_se:off_se + bt]
                    .rearrange("(t p) -> p t", p=128),
                    in_=se_all[:])
                nc.sync.dma_start(
                    out=red_in.ap()[0:sz_cand].rearrange("(p n) -> p n",
                                                         p=128),
                    in_=zeros_big[:, 0:sz_cand // 128])
                if rbp > rb:
                    nc.sync.dma_start(out=red_in.ap()[rb:rbp].unsqueeze(0),
                                      in_=zeros_big[0:1, 0:rbp - rb])
                pid = nc.partition_id()
                cand_view = red_in.ap()[0:sz_cand].rearrange(
                    "(c t p n) -> c t p n", c=n_cores, t=tcn, p=128,
                    n=CAND_N)
                for tci in range(tcn):
                    nc.sync.dma_start(out=cand_view[bass.ds(pid, 1), tci],
                                      in_=packs[:, tci])

                # ---------- AllReduce ----------
                if mock_cc:
                    nc.sync.dma_start(out=red_out.ap(), in_=red_in.ap())
                else:
                    nc.gpsimd.collective_compute(
                        "AllReduce", AL.add,
                        replica_groups=[list(range(n_cores))],
                        ins=[red_in.ap().opt()], outs=[red_out.ap().opt()])

            if stage >= 6:
                # ------- final merge / vote / loss (redundant per core) ---
                S_sb = sb.tile([128, tcn, C], F32, tag="S_sb")
                nc.sync.dma_start(
                    out=S_sb[:],
                    in_=red_out.ap()[off_s:off_s + sz_s]
                    .rearrange("(t p c) -> p t c", p=128, c=C))
                cands_sb = sb.tile([128, tcn, n_cores * CAND_N], F32,
                                   tag="cands_sb")
                cand_out_view = red_out.ap()[0:sz_cand].rearrange(
                    "(c t p n) -> t p c n", c=n_cores, p=128, n=CAND_N)
                for tci in range(tcn):
                    nc.sync.dma_start(
                        out=cands_sb[:, tci].rearrange("p (c n) -> p c n",
                                                       c=n_cores),
                        in_=cand_out_view[tci])
                se_sb = sb.tile([128, tcn], F32, tag="se_sb")
                nc.sync.dma_start(
                    out=se_sb[:],
                    in_=red_out.ap()[off_se:off_se + bt]
                    .rearrange("(t p) -> p t", p=128))
                cnt_bc = sb.tile([128, C], F32, tag="cnt_bc")
                nc.sync.dma_start(
                    out=cnt_bc[:],
                    in_=red_out.ap()[off_cnt:off_cnt + C]
                    .unsqueeze(0).partition_broadcast(128))

                ps_pack = sb.tile([128, tcn], F32, tag="ps_pack")
                n_pad_total = float(n_cores * n_pad_per_core)
                if stage >= 7:
                    for tci in range(tcn):
                        g8 = sb3.tile([128, 8], F32, tag="g8", name="g8")
                        nc.vector.max(g8[:], cands_sb[:, tci])
                        p5i = sb3.tile([128, 5], I32, tag="p5i", name="p5i")
                        nc.vector.tensor_copy(p5i[:], g8[:, 0:5])
                        enc5 = sb3.tile([128, 5], I32, tag="enc5",
                                        name="enc5")
                        nc.vector.tensor_scalar(enc5[:], p5i[:], 127, None,
                                                AL.bitwise_and)
                        labs5 = sb3.tile([128, 5], F32, tag="labs5",
                                         name="labs5")
                        nc.vector.tensor_scalar(labs5[:], enc5[:], -1, 127,
                                                AL.mult, AL.add)
                        cnt5 = sb3.tile([128, 5], F32, tag="cnt5",
                                        name="cnt5")
                        scr5 = sb3.tile([128, 5], F32, tag="scr5",
                                        name="scr5")
                        for k in range(5):
                            nc.vector.scalar_tensor_tensor(
                                out=scr5[:], in0=labs5[:],
                                scalar=labs5[:, k:k + 1], in1=labs5[:],
                                op0=AL.is_equal, op1=AL.bypass,
                                accum_out=cnt5[:, k:k + 1])
                        score = sb3.tile([128, 5], F32, tag="score",
                                         name="score")
                        nc.vector.scalar_tensor_tensor(
                            out=score[:], in0=cnt5[:], scalar=1024.0,
                            in1=labs5[:], op0=AL.mult, op1=AL.subtract)
                        nc.vector.tensor_scalar(score[:], score[:], 1023.0,
                                                None, AL.add)
                        best = sb3.tile([128, 1], F32, tag="best",
                                        name="best")
                        nc.vector.reduce_max(best[:], score[:], axis=AX.X)
                        besti = sb3.tile([128, 1], I32, tag="besti",
                                         name="besti")
                        nc.vector.tensor_copy(besti[:], best[:])
                        encb = sb3.tile([128, 1], I32, tag="encb",
                                        name="encb")
                        nc.vector.tensor_scalar(encb[:], besti[:], 1023,
                                                None, AL.bitwise_and)
                        pseudo = sb3.tile([128, 1], F32, tag="pseudo",
                                          name="pseudo")
                        nc.vector.tensor_scalar(pseudo[:], encb[:], -1,
                                                1023, AL.mult, AL.add)
                        pmask = sb3.tile([128, C], F32, tag="pmask",
                                         name="pmask")
                        nc.vector.tensor_scalar(pmask[:], ciota[:],
                                                pseudo[:, 0:1], None,
                                                AL.is_equal)
                        junk = sb3.tile([128, C], F32, tag="junk",
                                        name="junk")
                        spos = sb3.tile([128, 1], F32, tag="spos",
                                        name="spos")
                        nc.vector.scalar_tensor_tensor(
                            out=junk[:], in0=S_sb[:, tci], scalar=1.0,
                            in1=pmask[:], op0=AL.bypass, op1=AL.mult,
                            accum_out=spos[:])
                        cntp = sb3.tile([128, 1], F32, tag="cntp",
                                        name="cntp")
                        nc.vector.scalar_tensor_tensor(
                            out=junk[:], in0=cnt_bc[:], scalar=1.0,
                            in1=pmask[:], op0=AL.bypass, op1=AL.mult,
                            accum_out=cntp[:])
                        rc = sb3.tile([128, 1], F32, tag="rc", name="rc")
                        nc.vector.reciprocal(rc[:], cntp[:])
                        mp = sb3.tile([128, 1], F32, tag="mp", name="mp")
                        nc.vector.scalar_tensor_tensor(
                            out=mp[:], in0=spos[:], scalar=inv_tau,
                            in1=rc[:], op0=AL.mult, op1=AL.mult)
                        sec = sb3.tile([128, 1], F32, tag="sec", name="sec")
                        nc.vector.tensor_scalar(sec[:],
                                                se_sb[:, tci:tci + 1],
                                                -n_pad_total, None, AL.add)
                        lse = sb3.tile([128, 1], F32, tag="lse", name="lse")
                        nc.scalar.activation(lse[:], sec[:], AF.Ln)
                        nc.vector.tensor_sub(ps_pack[:, tci:tci + 1],
                                             lse[:], mp[:])

                if stage >= 8:
                    pm = psS.tile([128, C], F32, tag="s", name="pm")
                    pe(nc.tensor.matmul(pm[0:1, 0:tcn], lhsT=ones_f[:],
                                        rhs=ps_pack[:], start=True,
                                        stop=True,
                                        skip_group_check=SKIP_GC))
                    red_s = sb.tile([1, 1], F32, tag="red_s")
                    nc.vector.reduce_sum(red_s[:], pm[0:1, 0:tcn],
                                         axis=AX.X)
                    itv = sb.tile([1, 1], F32, tag="itv_sb")
                    nc.sync.dma_start(out=itv[:], in_=it_d.ap())
                    cf = sb.tile([1, 1], F32, tag="cf")
                    nc.vector.tensor_scalar(cf[:], itv[:], float(WARM_UP),
                                            COEFF / bt, AL.is_gt, AL.mult)
                    res = sb.tile([1, 1], F32, tag="res")
                    nc.vector.tensor_mul(res[:], red_s[:], cf[:])
                    nc.sync.dma_start(out=out_d.ap(), in_=res[:])
            if stage < 8:
                dres = sb.tile([1, 1], F32, tag="dres")
                nc.vector.memset(dres[:], 1.0)
                nc.sync.dma_start(out=out_d.ap(), in_=dres[:])

        for _rep in range(n_reps):
            with ExitStack() as ctx:
                _emit(ctx)

    nc.compile()
    return nc


def make_in_maps(features, source_labels, it, queue, queue_labels,
                 n_cores=NCORES, qsp=QSP):
    """Host-side sharding glue: substitute enqueued rows, shard + sort by
    label (layout permutation), build per-shard label/boundary tables."""
    features = np.asarray(features, dtype=np.float32)
    queue = np.asarray(queue, dtype=np.float32)
    src_lab = np.asarray(source_labels).astype(np.int64)
    q_lab = np.asarray(queue_labels).astype(np.int64)
    it_f = float(np.asarray(it))
    bs = src_lab.shape[0]
    qs_real = queue.shape[0] // n_cores

    src = features[:bs]
    tgt = np.ascontiguousarray(features[bs:])
    newq = queue.copy()
    newq[:bs] = src
    newl = q_lab.copy()
    newl[:bs] = src_lab

    nt = qsp // 128
    in_maps = []
    for c in range(n_cores):
        qs = newq[c * qs_real:(c + 1) * qs_real]
        ls = newl[c * qs_real:(c + 1) * qs_real]
        order = np.argsort(ls, kind="stable")
        q2 = np.zeros((qsp, D), np.float32)
        q2[:qs_real] = qs[order]
        l2 = np.full((qsp,), C, np.int64)
        l2[:qs_real] = ls[order]
        bounds = np.searchsorted(l2[:qs_real], np.arange(C),
                                 side="right").astype(np.float32)
        labT = np.ascontiguousarray(
            l2.reshape(nt, 128).T.astype(np.float32))
        in_maps.append({
            "tgt": tgt,
            "qshard": q2.astype(NP_BF16),
            "labT": labT,
            "bounds": bounds,
            "itv": np.array([[it_f]], np.float32),
        })
    return in_maps


_CACHED = {}


def _get_program():
    key = (NCORES, QSP, BT)
    if key not in _CACHED:
        _CACHED[key] = build_program(*key)
    return _CACHED[key]


def kernel(**inputs):
    nc = _get_program()
    in_maps = make_in_maps(inputs["features"], inputs["source_labels"],
                           inputs["it"], inputs["queue"],
                           inputs["queue_labels"])
    res = run_bass_kernel_spmd(nc, in_maps, core_ids=list(range(NCORES)))
    out = np.asarray(res.results[0]["outv"], np.float32).reshape(())
    return out


# revision 28
# speedup vs baseline: 1.3300x; 1.3300x over previous
"""MemSAC loss (retrieval kNN + masked log-softmax contrastive loss) on 8 Trainium2 cores.

Sharding: the 48000-slot memory queue is split 6000 rows/core (padded to
6016 = 47*128, pad rows zero / label 126). Queue rows are pre-sorted by
label per shard (host-side layout permutation) so a row's label is a step
function of its index, recoverable with compare-accumulate against a
126-entry boundary table.

Per core:
  - normalize targets (f32) -> bf16, PE-transpose to [D, T]
  - queue norms via fused multiply+accumulate; inverse norms fold into
    (a) diagonal-scaled PE transposes that produce the normalized
        transposed queue [D, Q] directly from the raw bf16 shard, and
    (b) the one-hot matrix (onehotn = onehot * invnorm) used for class sums
  - sim matmuls (bf16) into [128-target, 1024-col] PSUM superblocks; one
    ACT pass per superblock computes exp(sim/tau) (bf16 out) + row-sum
  - gpsimd packs (bf16-bits << 13 | column-index) into an int32 array whose
    f32 bit pattern is order-isomorphic to sim; one DVE max8 per target
    chunk yields the top-8 values AND indices (no max_index pass)
  - labels decoded from indices, candidates packed as
    (bf16bits << 7 | 127-label) -- exact in fp32
  - class sums W = qraw^T @ onehotn and counts via PE accumulation groups;
    S = tgt_n @ W^T
  - one AllReduce combines candidates (per-core slices of a zeroed
    buffer), S, sum-exp, counts
  - every core redundantly merges 64 candidates -> global top-5 ->
    majority vote (pairwise-equality count, score = cnt*1024+1023-label)
    -> pseudo label -> S[t,pseudo]/cnt[pseudo] -> loss.

PSUM accumulation groups are kept strictly contiguous on the PE via an
explicit instruction chain: the PE crashes (NRT_EXEC_UNIT_UNRECOVERABLE)
if two accumulation groups interleave.

kernel() takes FULL unsharded inputs and returns the FULL scalar output.
"""

import os
import sys

sys.path.insert(0, "/opt/trn_rl_repo")
os.environ.setdefault("MYCRO_LOCAL_CACHE", "1")

import numpy as np
from contextlib import ExitStack

import concourse.bass as bass
import concourse.bacc as bacc
import concourse.tile as tile
from concourse.tile import add_dep_helper
from concourse import mybir
from concourse.bass_utils import run_bass_kernel_spmd

AF = mybir.ActivationFunctionType
AL = mybir.AluOpType
AX = mybir.AxisListType
F32 = mybir.dt.float32
BF16 = mybir.dt.bfloat16
I32 = mybir.dt.int32
U16 = mybir.dt.uint16
NP_BF16 = mybir.dt.np(mybir.dt.bfloat16)

SKIP_GC = False
# ---- problem constants ----
D = 256
Q = 48000
C = 126          # n classes
BS = 512         # source batch
BT = 512         # target batch
TAU = 0.07
COEFF = 0.1
WARM_UP = 4000
NCORES = 8
QS = Q // NCORES            # 6000 real rows per core
NT = (QS + 127) // 128      # 47 tiles
QSP = NT * 128              # 6016 padded rows per core
NPAD = QSP - QS             # 16 pad rows per core
CAND_N = 8                  # candidates kept per core per row
SB = 1024                   # superblock width (2 PSUM banks)


def build_program(n_cores, qsp, bt, n_pad_per_core=None, stage=8,
                  mock_cc=False, n_reps=1):
    """Build the SPMD Bass program. Identical NEFF runs on all cores."""
    if n_pad_per_core is None:
        n_pad_per_core = NPAD
    nt = qsp // 128
    tcn = bt // 128
    nsb = (qsp + SB - 1) // SB
    sz_cand = n_cores * bt * CAND_N
    sz_s = bt * C
    off_s = sz_cand
    off_se = off_s + sz_s
    off_cnt = off_se + bt
    rb = off_cnt + C
    rbp = ((rb + 255) // 256) * 256
    inv_tau = 1.0 / TAU

    nc = bacc.Bacc("TRN2", target_bir_lowering=False, debug=False,
                   num_devices=n_cores)

    tgt_d = nc.dram_tensor("tgt", [bt, D], F32, kind="ExternalInput")
    qsh_d = nc.dram_tensor("qshard", [qsp, D], BF16, kind="ExternalInput")
    labT_d = nc.dram_tensor("labT", [128, nt], F32, kind="ExternalInput")
    bnd_d = nc.dram_tensor("bounds", [C], F32, kind="ExternalInput")
    it_d = nc.dram_tensor("itv", [1, 1], F32, kind="ExternalInput")
    out_d = nc.dram_tensor("outv", [1, 1], F32, kind="ExternalOutput")
    red_in = nc.dram_tensor("red_in", [rbp], F32)
    red_out = nc.dram_tensor("red_out", [rbp], F32,
                             addr_space="Shared" if n_cores > 4 else "Local")

    with tile.TileContext(nc) as tc:
        def _emit(ctx):
            sb = ctx.enter_context(tc.tile_pool(name="sb", bufs=1))
            sb2 = ctx.enter_context(tc.tile_pool(name="sb2", bufs=2))
            sb3 = ctx.enter_context(tc.tile_pool(name="sb3", bufs=3))
            psA = ctx.enter_context(tc.tile_pool(name="psA", bufs=2,
                                                 space="PSUM"))
            psW = ctx.enter_context(tc.tile_pool(name="psW", bufs=1,
                                                 space="PSUM"))
            psS = ctx.enter_context(tc.tile_pool(name="psS", bufs=1,
                                                 space="PSUM"))

            # PE group-contiguity chain (see module docstring)
            _pe_prev = [None]

            def pe(bi):
                if _pe_prev[0] is not None:
                    add_dep_helper(bi.ins, _pe_prev[0].ins, sync=False,
                                   reason="PE group contiguity")
                _pe_prev[0] = bi
                return bi

            def rsqrt_dve(dst, x, pool, w):
                # y = 1/sqrt(x) via 0x5f3759df magic + 2 Newton steps.
                # Tiny [128, w] tiles; keeps Sqrt out of the ACT table so
                # the exp set stays resident for the whole sim phase.
                xc = pool.tile([128, w], F32, tag="rsx", name="rsx",
                               bufs=4)
                nc.vector.tensor_scalar(xc[:], x, 1e-12, None, AL.max)
                x = xc[:]
                t0 = pool.tile([128, w], I32, tag="rs0", name="rs0", bufs=4)
                nc.vector.tensor_scalar(t0[:], x.bitcast(I32), 1, None,
                                        AL.logical_shift_right)
                y0 = pool.tile([128, w], I32, tag="rs1", name="rs1", bufs=4)
                nc.vector.tensor_scalar(y0[:], t0[:], -1, 0x5f3759df,
                                        AL.mult, AL.add)
                y = y0[:].bitcast(F32)
                for it_ in range(2):
                    t1 = pool.tile([128, w], F32, tag="rs2", name="rs2",
                                   bufs=4)
                    nc.vector.tensor_mul(t1[:], y, y)
                    nc.vector.tensor_mul(t1[:], t1[:], x)
                    nc.vector.tensor_scalar(t1[:], t1[:], -0.5, 1.5,
                                            AL.mult, AL.add)
                    if it_ == 1:
                        nc.vector.tensor_mul(dst, y, t1[:])
                    else:
                        yn = pool.tile([128, w], F32, tag="rs3", name="rs3",
                                       bufs=4)
                        nc.vector.tensor_mul(yn[:], y, t1[:])
                        y = yn[:]

            # ---------- constants ----------
            identi = sb.tile([128, 128], I32, tag="identi")
            nc.gpsimd.iota(identi[:], pattern=[[-1, 128]], base=0,
                           channel_multiplier=1)
            ident = sb.tile([128, 128], BF16, tag="ident")
            nc.vector.tensor_scalar(ident[:], identi[:], 0, None, AL.is_equal)

            ciota_i = sb.tile([128, C], I32, tag="ciota_i")
            nc.gpsimd.iota(ciota_i[:], pattern=[[1, C]], base=0,
                           channel_multiplier=0)
            ciota = sb.tile([128, C], F32, tag="ciota")
            nc.vector.tensor_copy(ciota[:], ciota_i[:])

            ones_bf = sb.tile([128, 1], BF16, tag="ones_bf")
            nc.vector.memset(ones_bf[:], 1.0)
            ones_f = sb.tile([128, 1], F32, tag="ones_f")
            nc.vector.memset(ones_f[:], 1.0)
            eps = sb.tile([128, 1], F32, tag="eps")
            nc.vector.memset(eps[:], 1e-24)
            zeros_big = sb.tile([128, max(sz_cand // 128, 256)], F32,
                                tag="zeros")
            nc.vector.memset(zeros_big[:], 0.0)

            e_bc = sb.tile([128, C], F32, tag="e_bc")
            nc.sync.dma_start(
                out=e_bc[:],
                in_=bnd_d.ap().unsqueeze(0).partition_broadcast(128))

            # ---------- target prep ----------
            tgtf = sb.tile([128, tcn, D], F32, tag="tgtf")
            nc.sync.dma_start(
                out=tgtf[:],
                in_=tgt_d.ap().rearrange("(c p) d -> p c d", p=128))
            ssq_t = sb.tile([128, tcn], F32, tag="ssq_t")
            sqscr = sb.tile([128, D], F32, tag="sqscr")
            for c_ in range(tcn):
                nc.vector.scalar_tensor_tensor(
                    out=sqscr[:], in0=tgtf[:, c_], scalar=1.0,
                    in1=tgtf[:, c_], op0=AL.bypass, op1=AL.mult,
                    accum_out=ssq_t[:, c_:c_ + 1])
            inv_t = sb.tile([128, tcn], F32, tag="inv_t")
            rsqrt_dve(inv_t[:], ssq_t[:], sb3, tcn)
            tgtn = sb.tile([128, tcn, D], BF16, tag="tgtn")
            for c_ in range(tcn):
                nc.scalar.activation(tgtn[:, c_], tgtf[:, c_], AF.Copy,
                                     bias=0.0, scale=inv_t[:, c_:c_ + 1])

            tgtT = [sb.tile([128, bt], BF16, tag=f"tgtT{kh}",
                            name=f"tgtT{kh}") for kh in range(2)]
            for c_ in range(tcn):
                for kh in range(2):
                    pt = psA.tile([128, 128], BF16, tag="mm", name="pt")
                    pe(nc.tensor.transpose(
                        pt[:, 0:128], tgtn[:, c_, kh * 128:(kh + 1) * 128],
                        ident[:]))
                    nc.scalar.activation(
                        tgtT[kh][:, c_ * 128:(c_ + 1) * 128], pt[:, 0:128],
                        AF.Copy)

            # ---------- queue shard load + norms ----------
            labT = sb.tile([128, nt], F32, tag="labT")
            nc.sync.dma_start(out=labT[:], in_=labT_d.ap())
            qsb = sb.tile([128, nt, D], BF16, tag="qsb")
            qview = qsh_d.ap().rearrange("(t p) d -> p t d", p=128)
            ssq_q = sb.tile([128, nt], F32, tag="ssq_q")
            sqscr2 = sb.tile([128, D], F32, tag="sqscr2")
            sqscr3 = sb.tile([128, D], BF16, tag="sqscr3")
            inv_q = sb.tile([128, nt], F32, tag="inv_q")
            oh_all = sb.tile([128, nt, C], BF16, tag="oh_all")
            qT = [sb.tile([128, qsp], BF16, tag=f"qT{kh}", name=f"qT{kh}")
                  for kh in range(2)]
            ngr = (nt + 3) // 4

            def prep_group(g):
                # load 4 tiles, norms for them, one-hot, scaled transposes
                ts0 = g * 4
                tn = min(4, nt - ts0)
                nc.sync.dma_start(out=qsb[:, ts0:ts0 + tn],
                                  in_=qview[:, ts0:ts0 + tn])
                for j in range(tn):
                    t = ts0 + j
                    if t % 2 == 0:
                        nc.vector.scalar_tensor_tensor(
                            out=sqscr2[:], in0=qsb[:, t], scalar=1.0,
                            in1=qsb[:, t], op0=AL.bypass, op1=AL.mult,
                            accum_out=ssq_q[:, t:t + 1])
                    else:
                        nc.scalar.activation(sqscr3[:], qsb[:, t],
                                             AF.Square,
                                             accum_out=ssq_q[:, t:t + 1])
                rsqrt_dve(inv_q[:, ts0:ts0 + tn],
                          ssq_q[:, ts0:ts0 + tn], sb3, tn)
                dgs = []
                for j in range(tn):
                    t = ts0 + j
                    nc.vector.tensor_scalar(oh_all[:, t], ciota[:],
                                            labT[:, t:t + 1],
                                            inv_q[:, t:t + 1],
                                            AL.is_equal, AL.mult)
                    dg = sb3.tile([128, 128], BF16, tag="diag",
                                  name="dg", bufs=8)
                    nc.vector.tensor_scalar(
                        dg[:], identi[:], 0, inv_q[:, t:t + 1],
                        AL.is_equal, AL.mult)
                    dgs.append(dg)
                for kh in range(2):
                    pq = psA.tile([128, 512], F32, tag="mm", name="pq")
                    for j in range(tn):
                        t = ts0 + j
                        pe(nc.tensor.matmul(
                            pq[:, j * 128:(j + 1) * 128],
                            lhsT=qsb[:, t, kh * 128:(kh + 1) * 128],
                            rhs=dgs[j][:], start=(j == 0),
                            stop=(j == tn - 1),
                            skip_group_check=SKIP_GC))
                    if kh == 0:
                        nc.scalar.activation(
                            qT[kh][:, ts0 * 128:(ts0 + tn) * 128],
                            pq[:, 0:tn * 128], AF.Copy)
                    else:
                        nc.vector.tensor_copy(
                            qT[kh][:, ts0 * 128:(ts0 + tn) * 128],
                            pq[:, 0:tn * 128])

            if stage >= 3:
                se_all = sb.tile([128, tcn], F32, tag="se_all")
                packs = sb.tile([128, tcn, CAND_N], F32, tag="packs")

                def emit_tc(tci, interleave_prep):
                    # vp[p, q] int32 = (bf16bits(exp(sim/tau)) << 16) | q :
                    # iota fills the low u16 halves (gpsimd), the ACT exp
                    # writes its bf16 output directly into the high halves.
                    # As f32 bit patterns these are positive, NaN-free, and
                    # ordered exactly like sim -- one max8 gives the top-8
                    # values AND their column indices.
                    vp = sb2.tile([128, qsp], I32, tag="vp", name="vp")
                    vp_u16 = vp[:].bitcast(U16).rearrange(
                        "p (q two) -> p q two", two=2)
                    nc.gpsimd.iota(vp_u16[:, :, 0], pattern=[[1, qsp]],
                                   base=0, channel_multiplier=0)
                    vp_hi = vp[:].bitcast(BF16).rearrange(
                        "p (q two) -> p q two", two=2)[:, :, 1]
                    parts = sb2.tile([128, nsb], F32, tag="parts",
                                     name="parts")
                    for sbi in range(nsb):
                        if interleave_prep:
                            while _prep_done[0] * 128 * 4 < min(
                                    (sbi + 1) * SB, nt * 128) or                                     (sbi == nsb - 1 and
                                     _prep_done[0] < ngr):
                                prep_group(_prep_done[0])
                                _prep_done[0] += 1
                        w = min(SB, qsp - sbi * SB)
                        ps = psA.tile([128, SB], F32, tag="mm", name="ps")
                        for half in range(0, w, 512):
                            n = min(512, w - half)
                            for kh in range(2):
                                col = sbi * SB + half
                                pe(nc.tensor.matmul(
                                    ps[:, half:half + n],
                                    lhsT=tgtT[kh][:, tci * 128:
                                                  (tci + 1) * 128],
                                    rhs=qT[kh][:, col:col + n],
                                    start=(kh == 0), stop=(kh == 1),
                                    skip_group_check=SKIP_GC))
                        nc.scalar.activation(
                            vp_hi[:, sbi * SB:sbi * SB + w], ps[:, 0:w],
                            AF.Exp, scale=inv_tau,
                            accum_out=parts[:, sbi:sbi + 1])
                    nc.vector.reduce_sum(se_all[:, tci:tci + 1], parts[:],
                                         axis=AX.X)
                    vp8 = sb2.tile([128, 8], F32, tag="vp8", name="vp8")
                    nc.vector.max(vp8[:], vp[:].bitcast(F32))
                    vp8i = vp8[:].bitcast(I32)
                    gix = sb2.tile([128, 8], I32, tag="gix", name="gix")
                    nc.vector.tensor_scalar(gix[:], vp8i, 65535, None,
                                            AL.bitwise_and)
                    gixf = sb2.tile([128, 8], F32, tag="gixf", name="gixf")
                    nc.vector.tensor_copy(gixf[:], gix[:])
                    lab8 = sb2.tile([128, 8], F32, tag="lab8", name="lab8")
                    scr126 = sb2.tile([128, C], F32, tag="scr126",
                                      name="scr126")
                    for k in range(8):
                        nc.vector.scalar_tensor_tensor(
                            out=scr126[:], in0=e_bc[:],
                            scalar=gixf[:, k:k + 1], in1=e_bc[:],
                            op0=AL.is_le, op1=AL.bypass,
                            accum_out=lab8[:, k:k + 1])
                    sh2 = sb2.tile([128, 8], I32, tag="sh2", name="sh2")
                    nc.vector.tensor_scalar(sh2[:], vp8i, 16, 7,
                                            AL.logical_shift_right,
                                            AL.logical_shift_left)
                    lab8i = sb2.tile([128, 8], I32, tag="lab8i",
                                     name="lab8i")
                    nc.vector.tensor_copy(lab8i[:], lab8[:])
                    packi = sb2.tile([128, 8], I32, tag="packi",
                                     name="packi")
                    nc.vector.scalar_tensor_tensor(
                        out=packi[:], in0=sh2[:], scalar=127, in1=lab8i[:],
                        op0=AL.add, op1=AL.subtract)
                    nc.vector.tensor_copy(packs[:, tci], packi[:])

                _prep_done = [0]
                for tci in range(tcn):
                    emit_tc(tci, interleave_prep=(tci == 0))

            if stage >= 4:
                # ------- class sums W / counts (contiguous groups) --------
                nrm_bf = sb.tile([128, nt], BF16, tag="nrm_bf")
                nc.vector.tensor_mul(nrm_bf[:], ssq_q[:], inv_q[:])
                wt_ps = [psW.tile([128, C], F32, tag=f"wt{kh}",
                                  name=f"wtps{kh}") for kh in range(2)]
                cnt_ps = psW.tile([1, C], F32, tag="cnt", name="cntps")
                for kh in range(2):
                    for t in range(nt):
                        pe(nc.tensor.matmul(
                            wt_ps[kh][:],
                            lhsT=qsb[:, t, kh * 128:(kh + 1) * 128],
                            rhs=oh_all[:, t], start=(t == 0),
                            stop=(t == nt - 1), skip_group_check=SKIP_GC))
                for t in range(nt):
                    pe(nc.tensor.matmul(
                        cnt_ps[:], lhsT=nrm_bf[:, t:t + 1],
                        rhs=oh_all[:, t], start=(t == 0),
                        stop=(t == nt - 1), skip_group_check=SKIP_GC))

                # ---------- per-class sums S = tgt_n @ W.T ----------
                wt_sb = [sb.tile([128, C], BF16, tag=f"wtsb{kh}",
                                 name=f"wtsb{kh}") for kh in range(2)]
                for kh in range(2):
                    nc.scalar.activation(wt_sb[kh][:], wt_ps[kh][:], AF.Copy)
                for tci in range(tcn):
                    pS = psS.tile([128, C], F32, tag="s", name="pS")
                    for kh in range(2):
                        pe(nc.tensor.matmul(
                            pS[:],
                            lhsT=tgtT[kh][:, tci * 128:(tci + 1) * 128],
                            rhs=wt_sb[kh][:], start=(kh == 0),
                            stop=(kh == 1), skip_group_check=SKIP_GC))
                    sout = sb3.tile([128, C], F32, tag="sout", name="sout")
                    nc.scalar.activation(sout[:], pS[:], AF.Copy)
                    nc.sync.dma_start(
                        out=red_in.ap()[off_s + tci * 128 * C:
                                        off_s + (tci + 1) * 128 * C]
                        .rearrange("(p c) -> p c", p=128),
                        in_=sout[:])

            if stage >= 5:
                # ---------- assemble reduce buffer ----------
                cnt_sb0 = sb.tile([1, C], F32, tag="cnt_sb0")
                nc.scalar.activation(cnt_sb0[:], cnt_ps[:], AF.Copy)
                nc.sync.dma_start(
                    out=red_in.ap()[off_cnt:off_cnt + C].unsqueeze(0),
                    in_=cnt_sb0[:])
                nc.sync.dma_start(
                    out=red_in.ap()[off_se:off_se + bt]
                    .rearrange("(t p) -> p t", p=128),
                    in_=se_all[:])
                nc.sync.dma_start(
                    out=red_in.ap()[0:sz_cand].rearrange("(p n) -> p n",
                                                         p=128),
                    in_=zeros_big[:, 0:sz_cand // 128])
                if rbp > rb:
                    nc.sync.dma_start(out=red_in.ap()[rb:rbp].unsqueeze(0),
                                      in_=zeros_big[0:1, 0:rbp - rb])
                pid = nc.partition_id()
                cand_view = red_in.ap()[0:sz_cand].rearrange(
                    "(c t p n) -> c t p n", c=n_cores, t=tcn, p=128,
                    n=CAND_N)
                for tci in range(tcn):
                    nc.sync.dma_start(out=cand_view[bass.ds(pid, 1), tci],
                                      in_=packs[:, tci])

                # ---------- AllReduce ----------
                if mock_cc:
                    nc.sync.dma_start(out=red_out.ap(), in_=red_in.ap())
                else:
                    nc.gpsimd.collective_compute(
                        "AllReduce", AL.add,
                        replica_groups=[list(range(n_cores))],
                        ins=[red_in.ap().opt()], outs=[red_out.ap().opt()])

            if stage >= 6:
                # ------- final merge / vote / loss (redundant per core) ---
                S_sb = sb.tile([128, tcn, C], F32, tag="S_sb")
                nc.sync.dma_start(
                    out=S_sb[:],
                    in_=red_out.ap()[off_s:off_s + sz_s]
                    .rearrange("(t p c) -> p t c", p=128, c=C))
                cands_sb = sb.tile([128, tcn, n_cores * CAND_N], F32,
                                   tag="cands_sb")
                cand_out_view = red_out.ap()[0:sz_cand].rearrange(
                    "(c t p n) -> t p c n", c=n_cores, p=128, n=CAND_N)
                for tci in range(tcn):
                    nc.sync.dma_start(
                        out=cands_sb[:, tci].rearrange("p (c n) -> p c n",
                                                       c=n_cores),
                        in_=cand_out_view[tci])
                se_sb = sb.tile([128, tcn], F32, tag="se_sb")
                nc.sync.dma_start(
                    out=se_sb[:],
                    in_=red_out.ap()[off_se:off_se + bt]
                    .rearrange("(t p) -> p t", p=128))
                cnt_bc = sb.tile([128, C], F32, tag="cnt_bc")
                nc.sync.dma_start(
                    out=cnt_bc[:],
                    in_=red_out.ap()[off_cnt:off_cnt + C]
                    .unsqueeze(0).partition_broadcast(128))

                ps_pack = sb.tile([128, tcn], F32, tag="ps_pack")
                n_pad_total = float(n_cores * n_pad_per_core)
                if stage >= 7:
                    for tci in range(tcn):
                        g8 = sb3.tile([128, 8], F32, tag="g8", name="g8")
                        nc.vector.max(g8[:], cands_sb[:, tci])
                        p5i = sb3.tile([128, 5], I32, tag="p5i", name="p5i")
                        nc.vector.tensor_copy(p5i[:], g8[:, 0:5])
                        enc5 = sb3.tile([128, 5], I32, tag="enc5",
                                        name="enc5")
                        nc.vector.tensor_scalar(enc5[:], p5i[:], 127, None,
                                                AL.bitwise_and)
                        labs5 = sb3.tile([128, 5], F32, tag="labs5",
                                         name="labs5")
                        nc.vector.tensor_scalar(labs5[:], enc5[:], -1, 127,
                                                AL.mult, AL.add)
                        cnt5 = sb3.tile([128, 5], F32, tag="cnt5",
                                        name="cnt5")
                        scr5 = sb3.tile([128, 5], F32, tag="scr5",
                                        name="scr5")
                        for k in range(5):
                            nc.vector.scalar_tensor_tensor(
                                out=scr5[:], in0=labs5[:],
                                scalar=labs5[:, k:k + 1], in1=labs5[:],
                                op0=AL.is_equal, op1=AL.bypass,
                                accum_out=cnt5[:, k:k + 1])
                        score = sb3.tile([128, 5], F32, tag="score",
                                         name="score")
                        nc.vector.scalar_tensor_tensor(
                            out=score[:], in0=cnt5[:], scalar=1024.0,
                            in1=labs5[:], op0=AL.mult, op1=AL.subtract)
                        nc.vector.tensor_scalar(score[:], score[:], 1023.0,
                                                None, AL.add)
                        best = sb3.tile([128, 1], F32, tag="best",
                                        name="best")
                        nc.vector.reduce_max(best[:], score[:], axis=AX.X)
                        besti = sb3.tile([128, 1], I32, tag="besti",
                                         name="besti")
                        nc.vector.tensor_copy(besti[:], best[:])
                        encb = sb3.tile([128, 1], I32, tag="encb",
                                        name="encb")
                        nc.vector.tensor_scalar(encb[:], besti[:], 1023,
                                                None, AL.bitwise_and)
                        pseudo = sb3.tile([128, 1], F32, tag="pseudo",
                                          name="pseudo")
                        nc.vector.tensor_scalar(pseudo[:], encb[:], -1,
                                                1023, AL.mult, AL.add)
                        pmask = sb3.tile([128, C], F32, tag="pmask",
                                         name="pmask")
                        nc.vector.tensor_scalar(pmask[:], ciota[:],
                                                pseudo[:, 0:1], None,
                                                AL.is_equal)
                        junk = sb3.tile([128, C], F32, tag="junk",
                                        name="junk")
                        spos = sb3.tile([128, 1], F32, tag="spos",
                                        name="spos")
                        nc.vector.scalar_tensor_tensor(
                            out=junk[:], in0=S_sb[:, tci], scalar=1.0,
                            in1=pmask[:], op0=AL.bypass, op1=AL.mult,
                            accum_out=spos[:])
                        cntp = sb3.tile([128, 1], F32, tag="cntp",
                                        name="cntp")
                        nc.vector.scalar_tensor_tensor(
                            out=junk[:], in0=cnt_bc[:], scalar=1.0,
                            in1=pmask[:], op0=AL.bypass, op1=AL.mult,
                            accum_out=cntp[:])
                        rc = sb3.tile([128, 1], F32, tag="rc", name="rc")
                        nc.vector.reciprocal(rc[:], cntp[:])
                        mp = sb3.tile([128, 1], F32, tag="mp", name="mp")
                        nc.vector.scalar_tensor_tensor(
                            out=mp[:], in0=spos[:], scalar=inv_tau,
                            in1=rc[:], op0=AL.mult, op1=AL.mult)
                        sec = sb3.tile([128, 1], F32, tag="sec", name="sec")
                        nc.vector.tensor_scalar(sec[:],
                                                se_sb[:, tci:tci + 1],
                                                -n_pad_total, None, AL.add)
                        lse = sb3.tile([128, 1], F32, tag="lse", name="lse")
                        nc.scalar.activation(lse[:], sec[:], AF.Ln)
                        nc.vector.tensor_sub(ps_pack[:, tci:tci + 1],
                                             lse[:], mp[:])

                if stage >= 8:
                    pm = psS.tile([128, C], F32, tag="s", name="pm")
                    pe(nc.tensor.matmul(pm[0:1, 0:tcn], lhsT=ones_f[:],
                                        rhs=ps_pack[:], start=True,
                                        stop=True,
                                        skip_group_check=SKIP_GC))
                    red_s = sb.tile([1, 1], F32, tag="red_s")
                    nc.vector.reduce_sum(red_s[:], pm[0:1, 0:tcn],
                                         axis=AX.X)
                    itv = sb.tile([1, 1], F32, tag="itv_sb")
                    nc.sync.dma_start(out=itv[:], in_=it_d.ap())
                    cf = sb.tile([1, 1], F32, tag="cf")
                    nc.vector.tensor_scalar(cf[:], itv[:], float(WARM_UP),
                                            COEFF / bt, AL.is_gt, AL.mult)
                    res = sb.tile([1, 1], F32, tag="res")
                    nc.vector.tensor_mul(res[:], red_s[:], cf[:])
                    nc.sync.dma_start(out=out_d.ap(), in_=res[:])
            if stage < 8:
                dres = sb.tile([1, 1], F32, tag="dres")
                nc.vector.memset(dres[:], 1.0)
                nc.sync.dma_start(out=out_d.ap(), in_=dres[:])

        for _rep in range(n_reps):
            with ExitStack() as ctx:
                _emit(ctx)

    nc.compile()
    return nc


def make_in_maps(features, source_labels, it, queue, queue_labels,
                 n_cores=NCORES, qsp=QSP):
    """Host-side sharding glue: substitute enqueued rows, shard + sort by
    label (layout permutation), build per-shard label/boundary tables."""
    features = np.asarray(features, dtype=np.float32)
    queue = np.asarray(queue, dtype=np.float32)
    src_lab = np.asarray(source_labels).astype(np.int64)
    q_lab = np.asarray(queue_labels).astype(np.int64)
    it_f = float(np.asarray(it))
    bs = src_lab.shape[0]
    qs_real = queue.shape[0] // n_cores

    src = features[:bs]
    tgt = np.ascontiguousarray(features[bs:])
    newq = queue.copy()
    newq[:bs] = src
    newl = q_lab.copy()
    newl[:bs] = src_lab

    nt = qsp // 128
    in_maps = []
    for c in range(n_cores):
        qs = newq[c * qs_real:(c + 1) * qs_real]
        ls = newl[c * qs_real:(c + 1) * qs_real]
        order = np.argsort(ls, kind="stable")
        q2 = np.zeros((qsp, D), np.float32)
        q2[:qs_real] = qs[order]
        l2 = np.full((qsp,), C, np.int64)
        l2[:qs_real] = ls[order]
        bounds = np.searchsorted(l2[:qs_real], np.arange(C),
                                 side="right").astype(np.float32)
        labT = np.ascontiguousarray(
            l2.reshape(nt, 128).T.astype(np.float32))
        in_maps.append({
            "tgt": tgt,
            "qshard": q2.astype(NP_BF16),
            "labT": labT,
            "bounds": bounds,
            "itv": np.array([[it_f]], np.float32),
        })
    return in_maps


_CACHED = {}


def _get_program():
    key = (NCORES, QSP, BT)
    if key not in _CACHED:
        _CACHED[key] = build_program(*key)
    return _CACHED[key]


def kernel(**inputs):
    nc = _get_program()
    in_maps = make_in_maps(inputs["features"], inputs["source_labels"],
                           inputs["it"], inputs["queue"],
                           inputs["queue_labels"])
    res = run_bass_kernel_spmd(nc, in_maps, core_ids=list(range(NCORES)))
    out = np.asarray(res.results[0]["outv"], np.float32).reshape(())
    return out
